# revision 1
# baseline (speedup 1.0000x reference)
"""Trainium2 Bass kernel: 16-head MHA (B=2, S=2048, E=1024) on 8 NeuronCores.

Sharding: core c = (batch b = c // 4, head-group g = c % 4); each core runs
4 heads of one batch (data parallel on B x tensor parallel on heads).  The
output projection is row-sharded: each core produces a partial [S, E] f32
output; the host sums the 4 head-group partials per batch and adds bo.

Device pipeline per core (all matmul operands bf16, fp32 PSUM accumulation):
  qT[d,m] = WqT.T-contract-e(xqT)     (weight-tile stationary, reused over
                                       all 4 m-chunks before switching)
  kT[d,n] = same
  vT[dv,n] = like q/k, then PE-transposed 128x128 into v_aug [n, dv] tiles
            carrying an extra ones column per head so the attention matmul
            also yields the softmax denominators
  scoresT[n,m] = kT-tile stationary (K=64; the two heads of a pair run in
            different PE row groups concurrently), qT moving; fully-masked
            causal columns are never computed (restricted matmul widths)
  probsT = exp(scoresT / sqrt(dk)) via one ACT per (j, head-pair); 0/1
            triangle multiply on diagonal tiles only (gpsimd)
  o_aug[dv+1,m] = v_aug stationary, probsT moving, accumulated over n-tiles,
            software-pipelined two steps behind the scores matmuls; row 64
            is sum(probs) = softmax denominator
  oT = o_aug[0:64] * bcast(1/denom): denom bounced through DRAM, re-read
            reshaped [128,4] so the reciprocal runs 128 lanes wide, bounced
            back, re-read with a step-0 partition AP as a [64,MC] broadcast
  out[m,e] partial = oT-tile stationary (serves both e-chunks), WoT moving
"""

import numpy as np
import ml_dtypes

B, S, E = 2, 2048, 1024
H, DK = 16, 64
NCORES = 8
G = 4                 # head-groups (tensor parallel degree)
NH = H // G           # heads per core = 4
DKH = NH * DK         # 256 head dims per core
P = 128
MC = 512              # m-chunk (psum bank width in f32)
NMC = S // MC         # 4 m-chunks
NT = S // P           # 16 n-tiles (and m-tiles)
ET = E // P           # 8 e-tiles
PAIRS = NH // 2       # 2 head pairs per core
BF16 = ml_dtypes.bfloat16
SCALE = float(1.0 / np.sqrt(np.float32(DK)))


def _build_program(chunk_ntiles, causal, bias_qk, bias_v):
    """Build the (SPMD, shared across all 8 cores) Bass program.

    chunk_ntiles[c] = number of 128-wide n-tiles to process for m-chunk c.
    causal: apply diagonal-tile masking (memset + tri multiply).
    """
    from contextlib import ExitStack

    import concourse.bass as bass
    import concourse.tile as tile
    from concourse import bacc, mybir
    from concourse.masks import make_identity

    f32 = mybir.dt.float32
    bf16 = mybir.dt.bfloat16
    Exp = mybir.ActivationFunctionType.Exp

    nc = bacc.Bacc(
        "TRN2",
        target_bir_lowering=False,
        debug=False,
        enable_asserts=False,
        num_devices=NCORES,
    )

    # ---- DRAM I/O ----
    xqT = nc.dram_tensor("xqT", [E, S], bf16, kind="ExternalInput").ap()
    xkT = nc.dram_tensor("xkT", [E, S], bf16, kind="ExternalInput").ap()
    xvT = nc.dram_tensor("xvT", [E, S], bf16, kind="ExternalInput").ap()
    wkqvT = nc.dram_tensor("wkqvT", [E, 3 * DKH], bf16, kind="ExternalInput").ap()
    woT = nc.dram_tensor("woT", [DKH, E], bf16, kind="ExternalInput").ap()
    dmask = nc.dram_tensor("dmask", [P, P], bf16, kind="ExternalInput").ap()
    if bias_qk:
        bqd = nc.dram_tensor("bq", [DKH, 1], f32, kind="ExternalInput").ap()
        bkd = nc.dram_tensor("bk", [DKH, 1], f32, kind="ExternalInput").ap()
    if bias_v:
        bvd = nc.dram_tensor("bv", [DKH, 1], f32, kind="ExternalInput").ap()
    out = nc.dram_tensor("out", [S, E], f32, kind="ExternalOutput").ap()

    with tile.TileContext(nc) as tc, ExitStack() as ctx:
        const = ctx.enter_context(tc.tile_pool(name="const", bufs=1))
        xpool = ctx.enter_context(tc.tile_pool(name="xpool", bufs=1))
        wpool = ctx.enter_context(tc.tile_pool(name="wpool", bufs=1))
        qkpool = ctx.enter_context(tc.tile_pool(name="qkpool", bufs=1))
        vpool = ctx.enter_context(tc.tile_pool(name="vpool", bufs=1))
        prpool = ctx.enter_context(tc.tile_pool(name="prpool", bufs=10))
        rcpool = ctx.enter_context(tc.tile_pool(name="rcpool", bufs=4))
        otpool = ctx.enter_context(tc.tile_pool(name="otpool", bufs=1))
        ostpool = ctx.enter_context(tc.tile_pool(name="ostpool", bufs=4))

        # weight tiles: wkqv packed on host into one [E, 3*DKH] tensor; one
        # DMA per 128-row block ([128, 768] = 1.5KB/partition contiguous).
        # DMA *issue* costs ~0.6us on the issuing engine regardless of size,
        # so few+large transfers matter more than fine-grained pacing.  The
        # first e-tile goes out first on the sync queue so the very first
        # projection LDWEIGHTS unblocks at ~1us.
        wkqv_sb = wpool.tile([P, ET, 3 * DKH], bf16, tag="wkqv")
        for i in range(ET):
            eng = nc.sync if i == 0 else nc.scalar
            eng.dma_start(out=wkqv_sb[:, i, :], in_=wkqvT[P * i : P * (i + 1), :])
        wk_sb = wkqv_sb[:, :, 0:DKH]
        wq_sb = wkqv_sb[:, :, DKH : 2 * DKH]
        wv_sb = wkqv_sb[:, :, 2 * DKH : 3 * DKH]
        wo_sb = wpool.tile([P, PAIRS, E], bf16, tag="wo")
        for p in range(PAIRS):
            nc.scalar.dma_start(out=wo_sb[:, p, :], in_=woT[P * p : P * (p + 1), :])

        if bias_qk:
            bq_sb = const.tile([P, PAIRS], f32, tag="bq")
            nc.sync.dma_start(out=bq_sb, in_=bqd.rearrange("(t p) o -> p (t o)", p=P))
            bk_sb = const.tile([P, PAIRS], f32, tag="bk")
            nc.sync.dma_start(out=bk_sb, in_=bkd.rearrange("(t p) o -> p (t o)", p=P))
        if bias_v:
            bv_sb = const.tile([P, PAIRS], f32, tag="bv")
            nc.sync.dma_start(out=bv_sb, in_=bvd.rearrange("(t p) o -> p (t o)", p=P))

        # x inputs: four 1MB DMAs per tensor (two e-tiles each, 4KB bursts) —
        # DMA issue costs ~0.6us/transfer on the issuing engine, so use few
        # large transfers, one tensor per DMA-capable engine in parallel
        xk_sb, xq_sb, xv_sb = [], [], []
        for x_sb, xT, engs, nm in (
            (xk_sb, xkT, (nc.sync,) * 4, "xk"),
            (xq_sb, xqT, (nc.scalar,) * 4, "xq"),
            (xv_sb, xvT, (nc.sync, nc.scalar, nc.sync, nc.scalar), "xv"),
        ):
            for i2 in range(ET // 2):
                pair = xpool.tile(
                    [P, 2, S], bf16, tag=f"{nm}pr{i2}", name=f"{nm}pair{i2}"
                )
                src = xT.rearrange("(t p) s -> p t s", p=P)[:, 2 * i2 : 2 * i2 + 2, :]
                if nm == "xk" and i2 == 0:
                    # split the first transfer so the first matmuls of the
                    # k-projection unblock after half the bytes
                    engs[i2].dma_start(out=pair[:, 0, :], in_=src[:, 0, :])
                    engs[i2].dma_start(out=pair[:, 1, :], in_=src[:, 1, :])
                else:
                    engs[i2].dma_start(out=pair, in_=src)
                x_sb.append(pair[:, 0, :])
                x_sb.append(pair[:, 1, :])

        # diagonal-mask constant: only needed once attention starts
        dmask_sb = const.tile([P, P], bf16, tag="dmask")
        nc.scalar.dma_start(out=dmask_sb, in_=dmask)

        # persistent activation tiles
        qT_sb = [qkpool.tile([P, S], bf16, tag=f"qT{p}", name=f"qT_sb{p}") for p in range(PAIRS)]
        kT_sb = [qkpool.tile([P, S], bf16, tag=f"kT{p}", name=f"kT_sb{p}") for p in range(PAIRS)]
        vaug_sb = [vpool.tile([P, NH, DK + 1], bf16, tag=f"va{j}", name=f"vaug_sb{j}") for j in range(NT)]
        oT_sb = [otpool.tile([P, S], bf16, tag=f"oT{p}", name=f"oT_sb{p}") for p in range(PAIRS)]

        # ---- stage 1: projections ----
        # q/k: weight-tile stationary, streamed over all 4 chunks (psum x4)
        # v:   x-tile stationary split into two row-group halves (concurrent)
        with tc.tile_pool(name="pj_ps", bufs=4, space="PSUM") as pjps:
            for dst, w_sb, x_sb, bias in (
                (kT_sb, wk_sb, xk_sb, bk_sb if bias_qk else None),
                (qT_sb, wq_sb, xq_sb, bq_sb if bias_qk else None),
            ):
                for p in range(PAIRS):
                    ps = [pjps.tile([P, MC], f32, tag="qk", name="ps_qk") for _ in range(NMC)]
                    for i in range(ET):
                        for c in range(NMC):
                            nc.tensor.matmul(
                                ps[c],
                                w_sb[:, i, P * p : P * (p + 1)],
                                x_sb[i][:, MC * c : MC * (c + 1)],
                                start=(i == 0),
                                stop=(i == ET - 1),
                            )
                    for c in range(NMC):
                        dslice = dst[p][:, MC * c : MC * (c + 1)]
                        if bias is not None:
                            nc.vector.tensor_scalar_add(dslice, ps[c], bias[:, p : p + 1])
                        else:
                            nc.scalar.copy(dslice, ps[c])
            # v: weight-block stationary producing vT [dv, n] (same efficient
            # shape as q/k), then PE-transpose 128x128 tiles into v_aug [n, dv]
            with tc.tile_pool(name="tp_ps", bufs=2, space="PSUM") as tpps:
                ident = const.tile([P, P], bf16, tag="ident")
                make_identity(nc, ident)
                vT_sb = [
                    qkpool.tile([P, S], bf16, tag=f"vT{db}", name=f"vT_sb{db}")
                    for db in range(PAIRS)
                ]
                for db in range(PAIRS):
                    ps = [pjps.tile([P, MC], f32, tag="qk", name="ps_v") for _ in range(NMC)]
                    for i in range(ET):
                        for c in range(NMC):
                            nc.tensor.matmul(
                                ps[c],
                                wv_sb[:, i, P * db : P * (db + 1)],
                                xv_sb[i][:, MC * c : MC * (c + 1)],
                                start=(i == 0),
                                stop=(i == ET - 1),
                            )
                    for c in range(NMC):
                        vslice = vT_sb[db][:, MC * c : MC * (c + 1)]
                        if bias_v:
                            nc.vector.tensor_scalar_add(vslice, ps[c], bv_sb[:, db : db + 1])
                        else:
                            nc.vector.tensor_copy(vslice, ps[c])
                    for j in range(NT):
                        pt = tpps.tile([P, P], bf16, tag="pt", name="pt_t")
                        nc.tensor.transpose(pt, vT_sb[db][:, P * j : P * (j + 1)], ident)
                        pt3 = pt.rearrange("n (h d) -> n h d", h=2)
                        nc.vector.tensor_copy(vaug_sb[j][:, 2 * db : 2 * db + 2, 0:DK], pt3)
                        if db == PAIRS - 1:
                            nc.vector.memset(vaug_sb[j][:, :, DK : DK + 1], 1.0)

        # ---- stage 2+3: attention with interleaved output projection ----
        # The oaps PSUM pool is shared between o_aug accumulators and output-
        # projection tiles (same shape) so 8 banks suffice while the outproj
        # matmuls fill the PE during the softmax epilogues.
        with (
            tc.tile_pool(name="sc_ps", bufs=2, space="PSUM") as scps,
            tc.tile_pool(name="oa_ps", bufs=4, space="PSUM") as oaps,
            tc.tile_pool(name="rc_dram", bufs=4, space="DRAM") as rcdram,
        ):
            for c in range(NMC):
                J = chunk_ntiles[c]
                for p in range(PAIRS):
                    oaug = [
                        oaps.tile([P, MC], f32, tag="oaug", name=f"oaug{h01}")
                        for h01 in range(2)
                    ]
                    probs_tiles = [None] * J

                    def scores_step(j):
                        # columns left of `off` in this m-chunk are fully
                        # masked for n-tile j: never compute/exp/consume them
                        off = P * (j - 4 * c) if (causal and j >= 4 * c) else 0
                        sc = scps.tile([P, 2 * MC], f32, tag="sc", name="sc_ps_t")
                        for h01 in range(2):
                            nc.tensor.matmul(
                                sc[:, MC * h01 + off : MC * (h01 + 1)],
                                kT_sb[p][64 * h01 : 64 * (h01 + 1), P * j : P * (j + 1)],
                                qT_sb[p][64 * h01 : 64 * (h01 + 1), MC * c + off : MC * (c + 1)],
                                start=True,
                                stop=True,
                            )
                        probs = prpool.tile([P, 2 * MC], bf16, tag="probs", name="probs_t")
                        sc3 = sc.rearrange("p (u m) -> p u m", u=2)
                        pr3 = probs.rearrange("p (u m) -> p u m", u=2)
                        nc.scalar.activation(
                            pr3[:, :, off:MC], sc3[:, :, off:MC], Exp, bias=0.0, scale=SCALE
                        )
                        if causal and j >= 4 * c:
                            for h01 in range(2):
                                base = MC * h01 + off
                                nc.gpsimd.tensor_mul(
                                    probs[:, base : base + P],
                                    probs[:, base : base + P],
                                    dmask_sb,
                                )
                        probs_tiles[j] = (probs, off)

                    def attnv_step(j):
                        probs, off = probs_tiles[j]
                        for h01 in range(2):
                            h = 2 * p + h01
                            nc.tensor.matmul(
                                oaug[h01][0 : DK + 1, off:MC],
                                vaug_sb[j][:, h, :],
                                probs[:, MC * h01 + off : MC * (h01 + 1)],
                                start=(j == 0),
                                stop=(j == J - 1),
                            )

                    # software pipeline: scores two steps ahead of attnV
                    for j in range(J):
                        scores_step(j)
                        if j >= 2:
                            attnv_step(j - 2)
                    attnv_step(J - 2)
                    attnv_step(J - 1)

                    # evict o_aug to SBUF right away (frees the PSUM slot for
                    # the next group), then normalize from the SBUF copy.
                    # The reciprocal of the [1, MC] denominator row would be a
                    # single-lane DVE op (~3.3us); instead bounce it through
                    # DRAM, re-read reshaped as [128, 4] (4 elems/lane), take
                    # the reciprocal there (~0.1us), bounce back, and re-read
                    # broadcast across 64 partitions.
                    osb, bcs = [], []
                    for h01 in range(2):
                        o = rcpool.tile([DK + 1, MC], f32, tag="osb", name="osb_t")
                        nc.vector.tensor_copy(o, oaug[h01][0 : DK + 1, :])
                        osb.append(o)
                    for h01 in range(2):
                        den_d = rcdram.tile([1, MC], f32, tag="den_d", name="den_d_t")
                        nc.sync.dma_start(out=den_d, in_=osb[h01][DK : DK + 1, :])
                        den_q = rcpool.tile([P, MC // P], f32, tag="den_q", name="den_q_t")
                        nc.sync.dma_start(
                            out=den_q,
                            in_=bass.AP(
                                tensor=den_d.tensor,
                                offset=den_d.offset,
                                ap=[[MC // P, P], [1, MC // P]],
                            ),
                        )
                        rcq = rcpool.tile([P, MC // P], f32, tag="rcq", name="rcq_t")
                        nc.vector.reciprocal(rcq, den_q)
                        rcd = rcdram.tile([1, MC], f32, tag="rcd", name="rcd_t")
                        nc.sync.dma_start(
                            out=bass.AP(
                                tensor=rcd.tensor,
                                offset=rcd.offset,
                                ap=[[MC // P, P], [1, MC // P]],
                            ),
                            in_=rcq,
                        )
                        bc = rcpool.tile([64, MC], f32, tag="bc", name="bc_t")
                        nc.sync.dma_start(
                            out=bc,
                            in_=bass.AP(
                                tensor=rcd.tensor,
                                offset=rcd.offset,
                                ap=[[0, 64]] + [list(a) for a in rcd.ap[1:]],
                            ),
                        )
                        bcs.append(bc)
                    for h01 in range(2):
                        nc.vector.tensor_mul(
                            oT_sb[p][64 * h01 : 64 * (h01 + 1), MC * c : MC * (c + 1)],
                            osb[h01][0:DK, :],
                            bcs[h01],
                        )

        # ---- stage 3: output projection ----
        with tc.tile_pool(name="op_ps", bufs=4, space="PSUM") as opps:
            for t in range(NT):
                op = [
                    opps.tile([P, MC], f32, tag="op", name="op_t")
                    for _ in range(E // MC)
                ]
                for p in range(PAIRS):
                    for ec in range(E // MC):
                        nc.tensor.matmul(
                            op[ec],
                            oT_sb[p][:, P * t : P * (t + 1)],
                            wo_sb[:, p, MC * ec : MC * (ec + 1)],
                            start=(p == 0),
                            stop=(p == PAIRS - 1),
                        )
                for ec in range(E // MC):
                    ost = ostpool.tile([P, MC], f32, tag="ost", name="ost_t")
                    nc.vector.tensor_copy(ost, op[ec])
                    (nc.sync if ec == 0 else nc.scalar).dma_start(
                        out=out[P * t : P * (t + 1), MC * ec : MC * (ec + 1)],
                        in_=ost,
                    )

    nc.compile()
    return nc


def _host_inputs(key, value, query, Wk, Wq, Wv, Wo, bq, bk, bv, bias_qk, bias_v):
    """Per-core input maps (host-side shard/transpose/cast — not timed)."""
    tri = np.triu(np.ones((P, P), np.float32)).astype(BF16)  # allowed: n<=m
    in_maps = []
    xT = {}
    for b in range(B):
        xT[("q", b)] = np.ascontiguousarray(query[b].T).astype(BF16)
        xT[("k", b)] = np.ascontiguousarray(key[b].T).astype(BF16)
        xT[("v", b)] = np.ascontiguousarray(value[b].T).astype(BF16)
    for c in range(NCORES):
        b, g = divmod(c, G)
        sl = slice(DKH * g, DKH * (g + 1))
        wkqv = np.concatenate(
            [Wk[sl].T, Wq[sl].T, Wv[sl].T], axis=1
        )  # [E, 3*DKH], column blocks K|Q|V
        m = {
            "xqT": xT[("q", b)],
            "xkT": xT[("k", b)],
            "xvT": xT[("v", b)],
            "wkqvT": np.ascontiguousarray(wkqv).astype(BF16),
            "woT": np.ascontiguousarray(Wo[:, sl].T).astype(BF16),
            "dmask": tri,
        }
        if bias_qk:
            m["bq"] = np.ascontiguousarray(bq[sl].astype(np.float32).reshape(DKH, 1))
            m["bk"] = np.ascontiguousarray(bk[sl].astype(np.float32).reshape(DKH, 1))
        if bias_v:
            m["bv"] = np.ascontiguousarray(bv[sl].astype(np.float32).reshape(DKH, 1))
        in_maps.append(m)
    return in_maps


def _numpy_fallback(key, value, query, mask, Wk, bk, Wq, bq, Wv, bv, Wo, bo):
    """Exact reference semantics in numpy (general-mask fallback)."""
    def proj(x, W, b):
        return x @ W.T + b

    k = proj(key, Wk, bk).reshape(B, S, H, DK).transpose(0, 2, 1, 3)
    q = proj(query, Wq, bq).reshape(B, S, H, DK).transpose(0, 2, 1, 3)
    v = proj(value, Wv, bv).reshape(B, S, H, DK).transpose(0, 2, 1, 3)
    scores = np.einsum("bhmd,bhnd->bhmn", q, k).astype(np.float32)
    scores = np.where(mask, scores, np.float32(-1e10)) * np.float32(SCALE)
    scores -= scores.max(axis=3, keepdims=True)
    e = np.exp(scores)
    attn = e / e.sum(axis=3, keepdims=True)
    o = np.einsum("bhmn,bhnv->bhmv", attn, v)
    o = o.transpose(0, 2, 1, 3).reshape(B, S, E)
    return (o @ Wo.T + bo).astype(np.float32)


_program_cache = {}


def kernel(key, value, query, mask, Wk, bk, Wq, bq, Wv, bv, Wo, bo):
    key = np.asarray(key, np.float32)
    value = np.asarray(value, np.float32)
    query = np.asarray(query, np.float32)
    mask = np.asarray(mask)
    Wk, bk = np.asarray(Wk, np.float32), np.asarray(bk, np.float32)
    Wq, bq = np.asarray(Wq, np.float32), np.asarray(bq, np.float32)
    Wv, bv = np.asarray(Wv, np.float32), np.asarray(bv, np.float32)
    Wo, bo = np.asarray(Wo, np.float32), np.asarray(bo, np.float32)

    m2 = mask.reshape(B, S, S) if mask.size == B * S * S else None
    causal = m2 is not None and all(
        np.array_equal(m2[b], np.tril(np.ones((S, S), bool))) for b in range(B)
    )
    allones = m2 is not None and bool(mask.all())
    if not causal and not allones:
        return _numpy_fallback(key, value, query, mask, Wk, bk, Wq, bq, Wv, bv, Wo, bo)

    if causal:
        chunk_ntiles = tuple(4 * (c + 1) for c in range(NMC))
    else:
        chunk_ntiles = tuple(NT for _ in range(NMC))

    bias_qk = bool(np.any(bq) or np.any(bk))
    bias_v = bool(np.any(bv))

    pkey = (chunk_ntiles, causal, bias_qk, bias_v)
    if pkey not in _program_cache:
        _program_cache[pkey] = _build_program(chunk_ntiles, causal, bias_qk, bias_v)
    nc = _program_cache[pkey]

    from concourse.bass_utils import run_bass_kernel_spmd

    in_maps = _host_inputs(key, value, query, Wk, Wq, Wv, Wo, bq, bk, bv, bias_qk, bias_v)
    res = run_bass_kernel_spmd(nc, in_maps, core_ids=list(range(NCORES)))

    outp = np.zeros((B, S, E), np.float32)
    for c in range(NCORES):
        outp[c // G] += res.results[c]["out"]
    outp += bo.astype(np.float32)
    return outp



# revision 3
# speedup vs baseline: 1.1767x; 1.1767x over previous
"""Trainium2 Bass kernel: 16-head MHA (B=2, S=2048, E=1024) on 8 NeuronCores.

Sharding: core c = (batch b = c // 4, head-group g = c % 4); each core runs
4 heads of one batch (data parallel on B x tensor parallel on heads).  The
output projection is row-sharded: each core produces a partial [S, E] bf16
output; the host sums the 4 head-group partials per batch (f32) and adds bo.

Device pipeline per core (all matmul operands bf16, fp32 PSUM accumulation).
The PE instruction stream is one continuous sequence ordered to never starve
(matmuls back-to-back keep the PE at its top p-state):

  k-proj -> q-proj -> scores(chunk0, both pairs)      [pre-issued: fills the
                                                       xv DMA-arrival gap]
  -> v[n-tiles 0:4] -> attnV(chunk0) -> v[4:8]
  -> attn(c1,p0) -> v[8:12] -> outproj(c0) -> attn(c1,p1) -> v[12:16]
  -> attn(c2,p0) -> outproj(c1) -> attn(c2,p1)
  -> attn(c3,p0) -> outproj(c2) -> attn(c3,p1) -> outproj(c3)

All input DMAs issue on the sync queue in priority order (wk/xk per e-tile
interleaved, then wqv, xq, xv) so each projection's operands stream in just
ahead of the PE.  v is produced directly in [n, dv] layout (stationary x
tile, moving wv) -- no PE transposes.  The softmax denominator rides as a
ones-column in v_aug; normalization bounces the denominator row through DRAM
(reshape to [128, 8] for a wide reciprocal, broadcast back with a 0-stride
partition AP), batched over both head-halves per group.
"""

import numpy as np
import ml_dtypes

B, S, E = 2, 2048, 1024
H, DK = 16, 64
NCORES = 8
G = 4                 # head-groups (tensor parallel degree)
NH = H // G           # heads per core = 4
DKH = NH * DK         # 256 head dims per core
P = 128
MC = 512              # m-chunk (psum bank width in f32)
NMC = S // MC         # 4 m-chunks
NT = S // P           # 16 n-tiles (and m-tiles)
ET = E // P           # 8 e-tiles
PAIRS = NH // 2       # 2 head pairs per core
BF16 = ml_dtypes.bfloat16
SCALE = float(1.0 / np.sqrt(np.float32(DK)))


def _build_program(chunk_ntiles, causal, bias_qk, bias_v):
    """Build the (SPMD, shared across all 8 cores) Bass program.

    chunk_ntiles[c] = number of 128-wide n-tiles to process for m-chunk c.
    causal: apply diagonal-tile masking (memset + tri multiply).
    """
    from contextlib import ExitStack

    import concourse.bass as bass
    import concourse.tile as tile
    from concourse import bacc, mybir

    f32 = mybir.dt.float32
    bf16 = mybir.dt.bfloat16
    Exp = mybir.ActivationFunctionType.Exp

    nc = bacc.Bacc(
        "TRN2",
        target_bir_lowering=False,
        debug=False,
        enable_asserts=False,
        num_devices=NCORES,
    )

    # ---- DRAM I/O ----
    xqT = nc.dram_tensor("xqT", [E, S], bf16, kind="ExternalInput").ap()
    xkT = nc.dram_tensor("xkT", [E, S], bf16, kind="ExternalInput").ap()
    xvT = nc.dram_tensor("xvT", [E, S], bf16, kind="ExternalInput").ap()
    wkT = nc.dram_tensor("wkT", [E, DKH], bf16, kind="ExternalInput").ap()
    wqvT = nc.dram_tensor("wqvT", [E, 2 * DKH], bf16, kind="ExternalInput").ap()
    woT = nc.dram_tensor("woT", [DKH, E], bf16, kind="ExternalInput").ap()
    dmask = nc.dram_tensor("dmask", [P, P], bf16, kind="ExternalInput").ap()
    if bias_qk:
        bqd = nc.dram_tensor("bq", [DKH, 1], f32, kind="ExternalInput").ap()
        bkd = nc.dram_tensor("bk", [DKH, 1], f32, kind="ExternalInput").ap()
    if bias_v:
        bvd = nc.dram_tensor("bv", [1, DKH], f32, kind="ExternalInput").ap()
    out = nc.dram_tensor("out", [S, E], bf16, kind="ExternalOutput").ap()

    with tile.TileContext(nc) as tc, ExitStack() as ctx:
        const = ctx.enter_context(tc.tile_pool(name="const", bufs=1))
        xpool = ctx.enter_context(tc.tile_pool(name="xpool", bufs=1))
        wpool = ctx.enter_context(tc.tile_pool(name="wpool", bufs=1))
        qkpool = ctx.enter_context(tc.tile_pool(name="qkpool", bufs=1))
        vpool = ctx.enter_context(tc.tile_pool(name="vpool", bufs=1))
        prpool = ctx.enter_context(tc.tile_pool(name="prpool", bufs=12))
        rcpool = ctx.enter_context(tc.tile_pool(name="rcpool", bufs=4))
        otpool = ctx.enter_context(tc.tile_pool(name="otpool", bufs=1))
        ostpool = ctx.enter_context(tc.tile_pool(name="ostpool", bufs=4))

        # ---- all input DMAs on the sync queue, in priority order ----
        # (one issuing queue => transfers start in issue order, so each
        # projection's operands arrive just ahead of the PE's need: the DMA
        # engines drain the queue FIFO at full HBM bandwidth.)
        wk_sb = wpool.tile([P, ET, DKH], bf16, tag="wk")
        wqv_sb = wpool.tile([P, ET, 2 * DKH], bf16, tag="wqv")
        wq_sb = wqv_sb[:, :, 0:DKH]
        wv_sb = wqv_sb[:, :, DKH : 2 * DKH]
        wo_sb = wpool.tile([P, PAIRS, E], bf16, tag="wo")
        xk_sb = xpool.tile([P, ET, S], bf16, tag="xk")
        xq_sb = xpool.tile([P, ET, S], bf16, tag="xq")
        xv_sb = xpool.tile([P, ET, S], bf16, tag="xv")
        xkr = xkT.rearrange("(t p) s -> p t s", p=P)
        xqr = xqT.rearrange("(t p) s -> p t s", p=P)
        xvr = xvT.rearrange("(t p) s -> p t s", p=P)
        wkr = wkT.rearrange("(t p) d -> p t d", p=P)
        wqvr = wqvT.rearrange("(t p) d -> p t d", p=P)
        for i in range(ET):
            nc.sync.dma_start(out=wk_sb[:, i, :], in_=wkr[:, i, :])
            nc.sync.dma_start(out=xk_sb[:, i, :], in_=xkr[:, i, :])
        for i in range(ET):
            nc.sync.dma_start(out=wqv_sb[:, i, :], in_=wqvr[:, i, :])
        for i in range(ET):
            nc.sync.dma_start(out=xq_sb[:, i, :], in_=xqr[:, i, :])
        for i in range(ET):
            nc.sync.dma_start(out=xv_sb[:, i, :], in_=xvr[:, i, :])
        for p in range(PAIRS):
            nc.sync.dma_start(out=wo_sb[:, p, :], in_=woT[P * p : P * (p + 1), :])
        dmask_sb = const.tile([P, P], bf16, tag="dmask")
        nc.sync.dma_start(out=dmask_sb, in_=dmask)
        if bias_qk:
            bq_sb = const.tile([P, PAIRS], f32, tag="bq")
            nc.sync.dma_start(out=bq_sb, in_=bqd.rearrange("(t p) o -> p (t o)", p=P))
            bk_sb = const.tile([P, PAIRS], f32, tag="bk")
            nc.sync.dma_start(out=bk_sb, in_=bkd.rearrange("(t p) o -> p (t o)", p=P))
        if bias_v:
            bv_sb = const.tile([P, 2 * DKH], f32, tag="bv")
            nc.sync.dma_start(
                out=bv_sb,
                in_=bass.AP(
                    tensor=bvd.tensor,
                    offset=bvd.offset,
                    ap=[[0, P], [1, DKH]],
                ),
            )

        # persistent activation tiles
        qT_sb = [qkpool.tile([P, S], bf16, tag=f"qT{p}", name=f"qT_sb{p}") for p in range(PAIRS)]
        kT_sb = [qkpool.tile([P, S], bf16, tag=f"kT{p}", name=f"kT_sb{p}") for p in range(PAIRS)]
        vaug_sb = [vpool.tile([P, NH, DK + 1], bf16, tag=f"va{j}", name=f"vaug_sb{j}") for j in range(NT)]
        oT_sb = [otpool.tile([P, S], bf16, tag=f"oT{p}", name=f"oT_sb{p}") for p in range(PAIRS)]

        # ---- stage 1: q/k projections (weight-tile stationary) ----
        with tc.tile_pool(name="pj_ps", bufs=4, space="PSUM") as pjps:
            for dst, w_sb, x_sb, bias in (
                (kT_sb, wk_sb, xk_sb, bk_sb if bias_qk else None),
                (qT_sb, wq_sb, xq_sb, bq_sb if bias_qk else None),
            ):
                for p in range(PAIRS):
                    ps = [pjps.tile([P, MC], f32, tag="qk", name="ps_qk") for _ in range(NMC)]
                    for i in range(ET):
                        for c in range(NMC):
                            nc.tensor.matmul(
                                ps[c],
                                w_sb[:, i, P * p : P * (p + 1)],
                                x_sb[:, i, MC * c : MC * (c + 1)],
                                start=(i == 0),
                                stop=(i == ET - 1),
                            )
                    for c in range(NMC):
                        dslice = dst[p][:, MC * c : MC * (c + 1)]
                        if bias is not None:
                            nc.vector.tensor_scalar_add(dslice, ps[c], bias[:, p : p + 1])
                        else:
                            nc.scalar.copy(dslice, ps[c])

        # ---- stages 2+3 pools: 4 (scores) + 2 (o_aug) + 2 (v-proj/outproj)
        # psum banks = 8 total.  v-projection n-tile blocks, attention groups
        # and per-chunk output projections are interleaved in ONE PE stream.
        with (
            tc.tile_pool(name="sc_ps", bufs=2, space="PSUM") as scps,
            tc.tile_pool(name="oa_ps", bufs=2, space="PSUM") as oaps,
            tc.tile_pool(name="mp_ps", bufs=2, space="PSUM") as mpps,
            tc.tile_pool(name="rc_dram", bufs=4, space="DRAM") as rcdram,
        ):

            def emit_vproj(j0, j1):
                # v directly in [n, dv] layout: stationary x-tile [e, n],
                # moving wv [e, dv] accumulated over e-tiles.
                for j in range(j0, j1):
                    ps = mpps.tile([P, MC], f32, tag="mp", name="ps_v")
                    for i in range(ET):
                        nc.tensor.matmul(
                            ps[:, 0:DKH],
                            xv_sb[:, i, P * j : P * (j + 1)],
                            wv_sb[:, i, :],
                            start=(i == 0),
                            stop=(i == ET - 1),
                        )
                    ps3 = ps[:, 0:DKH].rearrange("n (h d) -> n h d", h=NH)
                    if bias_v:
                        bv3 = bv_sb.rearrange("n (h d) -> n h d", h=NH)
                        nc.vector.tensor_add(vaug_sb[j][:, :, 0:DK], ps3, bv3)
                    else:
                        nc.vector.tensor_copy(vaug_sb[j][:, :, 0:DK], ps3)
                    nc.vector.memset(vaug_sb[j][:, :, DK : DK + 1], 1.0)

            class Group:
                """Attention for one (m-chunk c, head-pair p)."""

                def __init__(self, c, p):
                    self.c, self.p = c, p
                    self.J = chunk_ntiles[c]
                    self.probs = [None] * self.J
                    self.oaug = None

                def emit_scores(self, j):
                    c, p = self.c, self.p
                    off = P * (j - 4 * c) if (causal and j >= 4 * c) else 0
                    sc = scps.tile([P, 2 * MC], f32, tag="sc", name="sc_ps_t")
                    for h01 in range(2):
                        nc.tensor.matmul(
                            sc[:, MC * h01 + off : MC * (h01 + 1)],
                            kT_sb[p][64 * h01 : 64 * (h01 + 1), P * j : P * (j + 1)],
                            qT_sb[p][64 * h01 : 64 * (h01 + 1), MC * c + off : MC * (c + 1)],
                            start=True,
                            stop=True,
                        )
                    probs = prpool.tile([P, 2 * MC], bf16, tag="probs", name="probs_t")
                    sc3 = sc.rearrange("p (u m) -> p u m", u=2)
                    pr3 = probs.rearrange("p (u m) -> p u m", u=2)
                    nc.scalar.activation(
                        pr3[:, :, off:MC], sc3[:, :, off:MC], Exp, bias=0.0, scale=SCALE
                    )
                    if causal and j >= 4 * c:
                        for h01 in range(2):
                            base = MC * h01 + off
                            nc.gpsimd.tensor_mul(
                                probs[:, base : base + P],
                                probs[:, base : base + P],
                                dmask_sb,
                            )
                    self.probs[j] = (probs, off)

                def emit_attnv(self, j):
                    if self.oaug is None:
                        self.oaug = [
                            oaps.tile([P, MC], f32, tag="oaug", name=f"oaug{h01}")
                            for h01 in range(2)
                        ]
                    probs, off = self.probs[j]
                    for h01 in range(2):
                        h = 2 * self.p + h01
                        nc.tensor.matmul(
                            self.oaug[h01][0 : DK + 1, off:MC],
                            vaug_sb[j][:, h, :],
                            probs[:, MC * h01 + off : MC * (h01 + 1)],
                            start=(j == 0),
                            stop=(j == self.J - 1),
                        )

                def emit_norm(self):
                    # evict o_aug to SBUF (frees the PSUM slot), then divide
                    # rows 0:64 by row 64 (the ridden-along softmax denom).
                    # The [1, 2*MC] denom row would be a single-lane DVE op;
                    # bounce through DRAM: reshape to [128, 8] for a wide
                    # reciprocal, then re-read with a 0-stride partition AP
                    # as a [64, 2*MC] broadcast.  Both head-halves batched.
                    c, p = self.c, self.p
                    osb = []
                    for h01 in range(2):
                        o = rcpool.tile([DK + 1, MC], f32, tag="osb", name="osb_t")
                        nc.vector.tensor_copy(o, self.oaug[h01][0 : DK + 1, :])
                        osb.append(o)
                    den_d = rcdram.tile([2, MC], f32, tag="den_d", name="den_d_t")
                    for h01 in range(2):
                        nc.sync.dma_start(
                            out=den_d[h01 : h01 + 1, :], in_=osb[h01][DK : DK + 1, :]
                        )
                    den_q = rcpool.tile([P, 2 * MC // P], f32, tag="den_q", name="den_q_t")
                    nc.sync.dma_start(
                        out=den_q,
                        in_=bass.AP(
                            tensor=den_d.tensor,
                            offset=den_d.offset,
                            ap=[[2 * MC // P, P], [1, 2 * MC // P]],
                        ),
                    )
                    rcq = rcpool.tile([P, 2 * MC // P], f32, tag="rcq", name="rcq_t")
                    nc.vector.reciprocal(rcq, den_q)
                    rcd = rcdram.tile([1, 2 * MC], f32, tag="rcd", name="rcd_t")
                    nc.sync.dma_start(
                        out=bass.AP(
                            tensor=rcd.tensor,
                            offset=rcd.offset,
                            ap=[[2 * MC // P, P], [1, 2 * MC // P]],
                        ),
                        in_=rcq,
                    )
                    bc = rcpool.tile([64, 2 * MC], f32, tag="bc", name="bc_t")
                    nc.sync.dma_start(
                        out=bc,
                        in_=bass.AP(
                            tensor=rcd.tensor,
                            offset=rcd.offset,
                            ap=[[0, 64]] + [list(a) for a in rcd.ap[1:]],
                        ),
                    )
                    for h01 in range(2):
                        nc.vector.tensor_mul(
                            oT_sb[p][64 * h01 : 64 * (h01 + 1), MC * c : MC * (c + 1)],
                            osb[h01][0:DK, :],
                            bc[:, MC * h01 : MC * (h01 + 1)],
                        )

                def emit_pipelined(self):
                    # scores two steps ahead of attnV
                    J = self.J
                    for j in range(J):
                        self.emit_scores(j)
                        if j >= 2:
                            self.emit_attnv(j - 2)
                    self.emit_attnv(J - 2)
                    self.emit_attnv(J - 1)
                    self.emit_norm()

            def emit_outproj(c):
                # out[m, e] partial for the 4 m-tiles of chunk c; oT-tile
                # stationary serving both 512-wide e-chunks, wo moving.
                for t in range(4 * c, 4 * c + 4):
                    ost = ostpool.tile([P, E], bf16, tag="ost", name="ost_t")
                    for ec in range(E // MC):
                        op = mpps.tile([P, MC], f32, tag="mp", name="op_t")
                        for p in range(PAIRS):
                            nc.tensor.matmul(
                                op,
                                oT_sb[p][:, P * t : P * (t + 1)],
                                wo_sb[:, p, MC * ec : MC * (ec + 1)],
                                start=(p == 0),
                                stop=(p == PAIRS - 1),
                            )
                        nc.vector.tensor_copy(ost[:, MC * ec : MC * (ec + 1)], op)
                    nc.sync.dma_start(
                        out=out[P * t : P * (t + 1), :], in_=ost
                    )

            groups = {(c, p): Group(c, p) for c in range(NMC) for p in range(PAIRS)}

            if causal:
                # chunk-0 groups are tiny (J=4): pre-issue their scores so the
                # PE has work while xv streams in, then interleave v-proj
                # blocks / attention groups / per-chunk output projections.
                g00, g01 = groups[(0, 0)], groups[(0, 1)]
                for j in range(4):
                    g00.emit_scores(j)
                for j in range(4):
                    g01.emit_scores(j)
                emit_vproj(0, 4)
                for j in range(4):
                    g00.emit_attnv(j)
                g00.emit_norm()
                for j in range(4):
                    g01.emit_attnv(j)
                g01.emit_norm()
                emit_vproj(4, 8)
                groups[(1, 0)].emit_pipelined()
                emit_vproj(8, 12)
                emit_outproj(0)
                groups[(1, 1)].emit_pipelined()
                emit_vproj(12, 16)
                groups[(2, 0)].emit_pipelined()
                emit_outproj(1)
                groups[(2, 1)].emit_pipelined()
                groups[(3, 0)].emit_pipelined()
                emit_outproj(2)
                groups[(3, 1)].emit_pipelined()
                emit_outproj(3)
            else:
                emit_vproj(0, NT)
                for c in range(NMC):
                    for p in range(PAIRS):
                        groups[(c, p)].emit_pipelined()
                    if c > 0:
                        emit_outproj(c - 1)
                emit_outproj(NMC - 1)

    nc.compile()
    return nc


def _host_inputs(key, value, query, Wk, Wq, Wv, Wo, bq, bk, bv, bias_qk, bias_v):
    """Per-core input maps (host-side shard/transpose/cast — not timed)."""
    tri = np.triu(np.ones((P, P), np.float32)).astype(BF16)  # allowed: n<=m
    in_maps = []
    xT = {}
    for b in range(B):
        xT[("q", b)] = np.ascontiguousarray(query[b].T).astype(BF16)
        xT[("k", b)] = np.ascontiguousarray(key[b].T).astype(BF16)
        xT[("v", b)] = np.ascontiguousarray(value[b].T).astype(BF16)
    for c in range(NCORES):
        b, g = divmod(c, G)
        sl = slice(DKH * g, DKH * (g + 1))
        wqv = np.concatenate([Wq[sl].T, Wv[sl].T], axis=1)  # [E, 2*DKH]
        m = {
            "xqT": xT[("q", b)],
            "xkT": xT[("k", b)],
            "xvT": xT[("v", b)],
            "wkT": np.ascontiguousarray(Wk[sl].T).astype(BF16),
            "wqvT": np.ascontiguousarray(wqv).astype(BF16),
            "woT": np.ascontiguousarray(Wo[:, sl].T).astype(BF16),
            "dmask": tri,
        }
        if bias_qk:
            m["bq"] = np.ascontiguousarray(bq[sl].astype(np.float32).reshape(DKH, 1))
            m["bk"] = np.ascontiguousarray(bk[sl].astype(np.float32).reshape(DKH, 1))
        if bias_v:
            m["bv"] = np.ascontiguousarray(bv[sl].astype(np.float32).reshape(1, DKH))
        in_maps.append(m)
    return in_maps


def _numpy_fallback(key, value, query, mask, Wk, bk, Wq, bq, Wv, bv, Wo, bo):
    """Exact reference semantics in numpy (general-mask fallback)."""
    def proj(x, W, b):
        return x @ W.T + b

    k = proj(key, Wk, bk).reshape(B, S, H, DK).transpose(0, 2, 1, 3)
    q = proj(query, Wq, bq).reshape(B, S, H, DK).transpose(0, 2, 1, 3)
    v = proj(value, Wv, bv).reshape(B, S, H, DK).transpose(0, 2, 1, 3)
    scores = np.einsum("bhmd,bhnd->bhmn", q, k).astype(np.float32)
    scores = np.where(mask, scores, np.float32(-1e10)) * np.float32(SCALE)
    scores -= scores.max(axis=3, keepdims=True)
    e = np.exp(scores)
    attn = e / e.sum(axis=3, keepdims=True)
    o = np.einsum("bhmn,bhnv->bhmv", attn, v)
    o = o.transpose(0, 2, 1, 3).reshape(B, S, E)
    return (o @ Wo.T + bo).astype(np.float32)


_program_cache = {}


def kernel(key, value, query, mask, Wk, bk, Wq, bq, Wv, bv, Wo, bo):
    key = np.asarray(key, np.float32)
    value = np.asarray(value, np.float32)
    query = np.asarray(query, np.float32)
    mask = np.asarray(mask)
    Wk, bk = np.asarray(Wk, np.float32), np.asarray(bk, np.float32)
    Wq, bq = np.asarray(Wq, np.float32), np.asarray(bq, np.float32)
    Wv, bv = np.asarray(Wv, np.float32), np.asarray(bv, np.float32)
    Wo, bo = np.asarray(Wo, np.float32), np.asarray(bo, np.float32)

    m2 = mask.reshape(B, S, S) if mask.size == B * S * S else None
    causal = m2 is not None and all(
        np.array_equal(m2[b], np.tril(np.ones((S, S), bool))) for b in range(B)
    )
    allones = m2 is not None and bool(mask.all())
    if not causal and not allones:
        return _numpy_fallback(key, value, query, mask, Wk, bk, Wq, bq, Wv, bv, Wo, bo)

    if causal:
        chunk_ntiles = tuple(4 * (c + 1) for c in range(NMC))
    else:
        chunk_ntiles = tuple(NT for _ in range(NMC))

    bias_qk = bool(np.any(bq) or np.any(bk))
    bias_v = bool(np.any(bv))

    pkey = (chunk_ntiles, causal, bias_qk, bias_v)
    if pkey not in _program_cache:
        _program_cache[pkey] = _build_program(chunk_ntiles, causal, bias_qk, bias_v)
    nc = _program_cache[pkey]

    from concourse.bass_utils import run_bass_kernel_spmd

    in_maps = _host_inputs(key, value, query, Wk, Wq, Wv, Wo, bq, bk, bv, bias_qk, bias_v)
    res = run_bass_kernel_spmd(nc, in_maps, core_ids=list(range(NCORES)))

    outp = np.zeros((B, S, E), np.float32)
    for c in range(NCORES):
        outp[c // G] += np.asarray(res.results[c]["out"], np.float32)
    outp += bo.astype(np.float32)
    return outp


# revision 10
# speedup vs baseline: 1.2076x; 1.0262x over previous
"""Trainium2 Bass kernel: 16-head MHA (B=2, S=2048, E=1024) on 8 NeuronCores.

Sharding: core c = (batch b = c // 4, head-group g = c % 4); each core runs
4 heads of one batch (data parallel on B x tensor parallel on heads).  The
output projection is row-sharded: each core produces a partial [S, E] bf16
output; the host sums the 4 head-group partials per batch (f32) and adds bo.

Device pipeline per core (all matmul operands bf16, fp32 PSUM accumulation).
The PE instruction stream is one continuous sequence ordered to never starve
(matmuls back-to-back keep the PE at its top p-state):

  k-proj -> q-proj -> scores(chunk0, both pairs)      [pre-issued: fills the
                                                       xv DMA-arrival gap]
  -> v[n-tiles 0:4] -> attnV(chunk0) -> v[4:8]
  -> attn(c1,p0) -> v[8:12] -> outproj(c0) -> attn(c1,p1) -> v[12:16]
  -> attn(c2,p0) -> outproj(c1) -> attn(c2,p1)
  -> attn(c3,p0) -> outproj(c2) -> attn(c3,p1) -> outproj(c3)

All input DMAs issue on the sync queue in priority order (wk/xk per e-tile
interleaved, then wqv, xq, xv) so each projection's operands stream in just
ahead of the PE.  v is produced directly in [n, dv] layout (stationary x
tile, moving wv) -- no PE transposes.  The softmax denominator rides as a
ones-column in v_aug; normalization bounces the denominator row through DRAM
(reshape to [128, 8] for a wide reciprocal, broadcast back with a 0-stride
partition AP), batched over both head-halves per group.
"""

import numpy as np
import ml_dtypes

B, S, E = 2, 2048, 1024
H, DK = 16, 64
NCORES = 8
G = 4                 # head-groups (tensor parallel degree)
NH = H // G           # heads per core = 4
DKH = NH * DK         # 256 head dims per core
P = 128
MC = 512              # m-chunk (psum bank width in f32)
NMC = S // MC         # 4 m-chunks
NT = S // P           # 16 n-tiles (and m-tiles)
ET = E // P           # 8 e-tiles
PAIRS = NH // 2       # 2 head pairs per core
BF16 = ml_dtypes.bfloat16
SCALE = float(1.0 / np.sqrt(np.float32(DK)))


def _build_program(chunk_ntiles, causal, bias_qk, bias_v):
    """Build the (SPMD, shared across all 8 cores) Bass program.

    chunk_ntiles[c] = number of 128-wide n-tiles to process for m-chunk c.
    causal: apply diagonal-tile masking (memset + tri multiply).
    """
    from contextlib import ExitStack

    import concourse.bass as bass
    import concourse.tile as tile
    from concourse import bacc, mybir
    from concourse.masks import make_identity

    f32 = mybir.dt.float32
    bf16 = mybir.dt.bfloat16
    Exp = mybir.ActivationFunctionType.Exp

    nc = bacc.Bacc(
        "TRN2",
        target_bir_lowering=False,
        debug=False,
        enable_asserts=False,
        num_devices=NCORES,
    )

    # ---- DRAM I/O ----
    xqT = nc.dram_tensor("xqT", [E, S], bf16, kind="ExternalInput").ap()
    xkT = nc.dram_tensor("xkT", [E, S], bf16, kind="ExternalInput").ap()
    xvT = nc.dram_tensor("xvT", [E, S], bf16, kind="ExternalInput").ap()
    wkT = nc.dram_tensor("wkT", [E, DKH], bf16, kind="ExternalInput").ap()
    wqvT = nc.dram_tensor("wqvT", [E, 2 * DKH], bf16, kind="ExternalInput").ap()
    woT = nc.dram_tensor("woT", [DKH, E], bf16, kind="ExternalInput").ap()
    dmask = nc.dram_tensor("dmask", [P, P], bf16, kind="ExternalInput").ap()
    if bias_qk:
        bqd = nc.dram_tensor("bq", [DKH, 1], f32, kind="ExternalInput").ap()
        bkd = nc.dram_tensor("bk", [DKH, 1], f32, kind="ExternalInput").ap()
    if bias_v:
        bvd = nc.dram_tensor("bv", [1, DKH], f32, kind="ExternalInput").ap()
    out = nc.dram_tensor("out", [S, E], bf16, kind="ExternalOutput").ap()

    with tile.TileContext(nc) as tc, ExitStack() as ctx:
        const = ctx.enter_context(tc.tile_pool(name="const", bufs=1))
        xpool = ctx.enter_context(tc.tile_pool(name="xpool", bufs=1))
        wpool = ctx.enter_context(tc.tile_pool(name="wpool", bufs=1))
        qkpool = ctx.enter_context(tc.tile_pool(name="qkpool", bufs=1))
        vpool = ctx.enter_context(tc.tile_pool(name="vpool", bufs=1))
        prpool = ctx.enter_context(tc.tile_pool(name="prpool", bufs=12))
        rcpool = ctx.enter_context(tc.tile_pool(name="rcpool", bufs=4))
        otpool = ctx.enter_context(tc.tile_pool(name="otpool", bufs=1))
        ostpool = ctx.enter_context(tc.tile_pool(name="ostpool", bufs=4))

        # ---- all input DMAs on the sync queue, in priority order ----
        # (one issuing queue => transfers start in issue order, so each
        # projection's operands arrive just ahead of the PE's need: the DMA
        # engines drain the queue FIFO at full HBM bandwidth.)
        wk_sb = wpool.tile([P, ET, DKH], bf16, tag="wk")
        wqv_sb = wpool.tile([P, ET, 2 * DKH], bf16, tag="wqv")
        wq_sb = wqv_sb[:, :, 0:DKH]
        wv_sb = wqv_sb[:, :, DKH : 2 * DKH]
        wo_sb = wpool.tile([P, PAIRS, E], bf16, tag="wo")
        xk_sb = xpool.tile([P, ET, S], bf16, tag="xk")
        xq_sb = xpool.tile([P, ET, S], bf16, tag="xq")
        xv_sb = xpool.tile([P, ET, S], bf16, tag="xv")
        xkr = xkT.rearrange("(t p) s -> p t s", p=P)
        xqr = xqT.rearrange("(t p) s -> p t s", p=P)
        xvr = xvT.rearrange("(t p) s -> p t s", p=P)
        wkr = wkT.rearrange("(t p) d -> p t d", p=P)
        wqvr = wqvT.rearrange("(t p) d -> p t d", p=P)
        for i in range(ET):
            nc.sync.dma_start(out=wk_sb[:, i, :], in_=wkr[:, i, :])
            nc.sync.dma_start(out=xk_sb[:, i, :], in_=xkr[:, i, :])
        for i in range(ET):
            nc.sync.dma_start(out=wqv_sb[:, i, :], in_=wqvr[:, i, :])
        for i in range(ET):
            nc.sync.dma_start(out=xq_sb[:, i, :], in_=xqr[:, i, :])
        for i in range(ET):
            nc.sync.dma_start(out=xv_sb[:, i, :], in_=xvr[:, i, :])
        for p in range(PAIRS):
            nc.sync.dma_start(out=wo_sb[:, p, :], in_=woT[P * p : P * (p + 1), :])
        dmask_sb = const.tile([P, P], bf16, tag="dmask")
        nc.sync.dma_start(out=dmask_sb, in_=dmask)
        ident_f32 = const.tile([P, P], f32, tag="ident")
        make_identity(nc, ident_f32)
        if bias_qk:
            bq_sb = const.tile([P, PAIRS], f32, tag="bq")
            nc.sync.dma_start(out=bq_sb, in_=bqd.rearrange("(t p) o -> p (t o)", p=P))
            bk_sb = const.tile([P, PAIRS], f32, tag="bk")
            nc.sync.dma_start(out=bk_sb, in_=bkd.rearrange("(t p) o -> p (t o)", p=P))
        if bias_v:
            bv_sb = const.tile([P, 2 * DKH], f32, tag="bv")
            nc.sync.dma_start(
                out=bv_sb,
                in_=bass.AP(
                    tensor=bvd.tensor,
                    offset=bvd.offset,
                    ap=[[0, P], [1, DKH]],
                ),
            )

        # persistent activation tiles
        qT_sb = [qkpool.tile([P, S], bf16, tag=f"qT{p}", name=f"qT_sb{p}") for p in range(PAIRS)]
        kT_sb = [qkpool.tile([P, S], bf16, tag=f"kT{p}", name=f"kT_sb{p}") for p in range(PAIRS)]
        vaug_sb = [vpool.tile([P, NH, DK + 1], bf16, tag=f"va{j}", name=f"vaug_sb{j}") for j in range(NT)]
        oT_sb = [otpool.tile([P, S], bf16, tag=f"oT{p}", name=f"oT_sb{p}") for p in range(PAIRS)]

        # ---- stage 1: q/k projections (weight-tile stationary) ----
        with tc.tile_pool(name="pj_ps", bufs=4, space="PSUM") as pjps:
            for dst, w_sb, x_sb, bias in (
                (kT_sb, wk_sb, xk_sb, bk_sb if bias_qk else None),
                (qT_sb, wq_sb, xq_sb, bq_sb if bias_qk else None),
            ):
                for p in range(PAIRS):
                    ps = [pjps.tile([P, MC], f32, tag="qk", name="ps_qk") for _ in range(NMC)]
                    for i in range(ET):
                        for c in range(NMC):
                            nc.tensor.matmul(
                                ps[c],
                                w_sb[:, i, P * p : P * (p + 1)],
                                x_sb[:, i, MC * c : MC * (c + 1)],
                                start=(i == 0),
                                stop=(i == ET - 1),
                            )
                    for c in range(NMC):
                        dslice = dst[p][:, MC * c : MC * (c + 1)]
                        if bias is not None:
                            nc.vector.tensor_scalar_add(dslice, ps[c], bias[:, p : p + 1])
                        else:
                            nc.scalar.copy(dslice, ps[c])

        # ---- stages 2+3 pools: 4 (scores) + 2 (o_aug) + 2 (v-proj/outproj)
        # psum banks = 8 total.  v-projection n-tile blocks, attention groups
        # and per-chunk output projections are interleaved in ONE PE stream.
        with (
            tc.tile_pool(name="sc_ps", bufs=2, space="PSUM") as scps,
            tc.tile_pool(name="oa_ps", bufs=2, space="PSUM") as oaps,
            tc.tile_pool(name="mp_ps", bufs=2, space="PSUM") as mpps,
            tc.tile_pool(name="rc_dram", bufs=4, space="DRAM") as rcdram,
        ):

            def emit_vproj(j0, j1):
                # v directly in [n, dv] layout: stationary x-tile [e, n],
                # moving wv [e, dv] accumulated over e-tiles.
                for j in range(j0, j1):
                    ps = mpps.tile([P, MC], f32, tag="mp", name="ps_v")
                    for i in range(ET):
                        nc.tensor.matmul(
                            ps[:, 0:DKH],
                            xv_sb[:, i, P * j : P * (j + 1)],
                            wv_sb[:, i, :],
                            start=(i == 0),
                            stop=(i == ET - 1),
                        )
                    ps3 = ps[:, 0:DKH].rearrange("n (h d) -> n h d", h=NH)
                    if bias_v:
                        bv3 = bv_sb.rearrange("n (h d) -> n h d", h=NH)
                        nc.vector.tensor_add(vaug_sb[j][:, :, 0:DK], ps3, bv3)
                    else:
                        nc.vector.tensor_copy(vaug_sb[j][:, :, 0:DK], ps3)
                    nc.vector.memset(vaug_sb[j][:, :, DK : DK + 1], 1.0)

            class Group:
                """Attention for one (m-chunk c, head-pair p)."""

                def __init__(self, c, p):
                    self.c, self.p = c, p
                    self.J = chunk_ntiles[c]
                    self.probs = [None] * self.J
                    self.oaug = None

                def emit_scores(self, j):
                    c, p = self.c, self.p
                    off = P * (j - 4 * c) if (causal and j >= 4 * c) else 0
                    sc = scps.tile([P, 2 * MC], f32, tag="sc", name="sc_ps_t")
                    for h01 in range(2):
                        nc.tensor.matmul(
                            sc[:, MC * h01 + off : MC * (h01 + 1)],
                            kT_sb[p][64 * h01 : 64 * (h01 + 1), P * j : P * (j + 1)],
                            qT_sb[p][64 * h01 : 64 * (h01 + 1), MC * c + off : MC * (c + 1)],
                            start=True,
                            stop=True,
                        )
                    probs = prpool.tile([P, 2 * MC], bf16, tag="probs", name="probs_t")
                    sc3 = sc.rearrange("p (u m) -> p u m", u=2)
                    pr3 = probs.rearrange("p (u m) -> p u m", u=2)
                    nc.scalar.activation(
                        pr3[:, :, off:MC], sc3[:, :, off:MC], Exp, bias=0.0, scale=SCALE
                    )
                    if causal and j >= 4 * c:
                        for h01 in range(2):
                            base = MC * h01 + off
                            nc.gpsimd.tensor_mul(
                                probs[:, base : base + P],
                                probs[:, base : base + P],
                                dmask_sb,
                            )
                    self.probs[j] = (probs, off)

                def emit_attnv(self, j):
                    if self.oaug is None:
                        self.oaug = [
                            oaps.tile([P, MC], f32, tag="oaug", name=f"oaug{h01}")
                            for h01 in range(2)
                        ]
                    probs, off = self.probs[j]
                    for h01 in range(2):
                        h = 2 * self.p + h01
                        nc.tensor.matmul(
                            self.oaug[h01][0 : DK + 1, off:MC],
                            vaug_sb[j][:, h, :],
                            probs[:, MC * h01 + off : MC * (h01 + 1)],
                            start=(j == 0),
                            stop=(j == self.J - 1),
                        )

                def emit_norm(self):
                    # evict o_aug to SBUF (frees the PSUM slot), then divide
                    # rows 0:64 by row 64 (the ridden-along softmax denom).
                    # The [1, 2*MC] denom row would be a single-lane DVE op;
                    # bounce through DRAM: reshape to [128, 8] for a wide
                    # reciprocal, then re-read with a 0-stride partition AP
                    # as a [64, 2*MC] broadcast.  Both head-halves batched.
                    c, p = self.c, self.p
                    osb = []
                    for h01 in range(2):
                        o = rcpool.tile([DK + 1, MC], f32, tag="osb", name="osb_t")
                        nc.vector.tensor_copy(o, self.oaug[h01][0 : DK + 1, :])
                        osb.append(o)
                    den_d = rcdram.tile([2, MC], f32, tag="den_d", name="den_d_t")
                    for h01 in range(2):
                        nc.sync.dma_start(
                            out=den_d[h01 : h01 + 1, :], in_=osb[h01][DK : DK + 1, :]
                        )
                    den_q = rcpool.tile([P, 2 * MC // P], f32, tag="den_q", name="den_q_t")
                    nc.sync.dma_start(
                        out=den_q,
                        in_=bass.AP(
                            tensor=den_d.tensor,
                            offset=den_d.offset,
                            ap=[[2 * MC // P, P], [1, 2 * MC // P]],
                        ),
                    )
                    rcq = rcpool.tile([P, 2 * MC // P], f32, tag="rcq", name="rcq_t")
                    nc.vector.reciprocal(rcq, den_q)
                    rcd = rcdram.tile([1, 2 * MC], f32, tag="rcd", name="rcd_t")
                    nc.sync.dma_start(
                        out=bass.AP(
                            tensor=rcd.tensor,
                            offset=rcd.offset,
                            ap=[[2 * MC // P, P], [1, 2 * MC // P]],
                        ),
                        in_=rcq,
                    )
                    bc = rcpool.tile([64, 2 * MC], f32, tag="bc", name="bc_t")
                    nc.sync.dma_start(
                        out=bc,
                        in_=bass.AP(
                            tensor=rcd.tensor,
                            offset=rcd.offset,
                            ap=[[0, 64]] + [list(a) for a in rcd.ap[1:]],
                        ),
                    )
                    for h01 in range(2):
                        nc.vector.tensor_mul(
                            oT_sb[p][64 * h01 : 64 * (h01 + 1), MC * c : MC * (c + 1)],
                            osb[h01][0:DK, :],
                            bc[:, MC * h01 : MC * (h01 + 1)],
                        )

                def emit_norm_transpose(self):
                    # all-on-chip normalization (no DMA hops): PE-transpose
                    # each [65, 128] block of o_aug so the denominator becomes
                    # a [128, 1] column, wide-reciprocal it, per-partition
                    # multiply, transpose back.  Used for the final group,
                    # whose normalization latency is tail-exposed.
                    c, p = self.c, self.p
                    osb = []
                    for h01 in range(2):
                        o = rcpool.tile([DK + 1, MC], f32, tag="osb", name="osb_t")
                        nc.vector.tensor_copy(o, self.oaug[h01][0 : DK + 1, :])
                        osb.append(o)
                    for h01 in range(2):
                        for sub in range(MC // P):
                            ot = mpps.tile([P, MC], f32, tag="mp", name="ot_t")
                            nc.tensor.transpose(
                                ot[:, 0 : DK + 1],
                                osb[h01][:, P * sub : P * (sub + 1)],
                                ident_f32[0 : DK + 1, 0 : DK + 1],
                            )
                            rc = rcpool.tile([P, 1], f32, tag="rc_c", name="rc_c_t")
                            nc.vector.reciprocal(rc, ot[:, DK : DK + 1])
                            otn = rcpool.tile([P, DK], f32, tag="otn", name="otn_t")
                            nc.vector.tensor_scalar_mul(otn, ot[:, 0:DK], rc)
                            pt = mpps.tile([P, MC], f32, tag="mp", name="pt_t")
                            nc.tensor.transpose(
                                pt[0:DK, 0:P], otn, ident_f32
                            )
                            nc.vector.tensor_copy(
                                oT_sb[p][
                                    64 * h01 : 64 * h01 + DK,
                                    MC * c + P * sub : MC * c + P * (sub + 1),
                                ],
                                pt[0:DK, 0:P],
                            )

            def emit_outproj(c, tail=False):
                # out[m, e] partial for the 4 m-tiles of chunk c; oT-tile
                # stationary serving both 512-wide e-chunks, wo moving.
                for t in range(4 * c, 4 * c + 4):
                    ost = ostpool.tile([P, E], bf16, tag="ost", name="ost_t")
                    for ec in range(E // MC):
                        op = mpps.tile([P, MC], f32, tag="mp", name="op_t")
                        for p in range(PAIRS):
                            nc.tensor.matmul(
                                op,
                                oT_sb[p][:, P * t : P * (t + 1)],
                                wo_sb[:, p, MC * ec : MC * (ec + 1)],
                                start=(p == 0),
                                stop=(p == PAIRS - 1),
                            )
                        nc.vector.tensor_copy(ost[:, MC * ec : MC * (ec + 1)], op)
                    # the scalar queue is free once the exps are done, so the
                    # tail chunk's writes alternate queues to drain faster
                    eng = nc.scalar if (tail and t % 2) else nc.sync
                    eng.dma_start(out=out[P * t : P * (t + 1), :], in_=ost)

            groups = {(c, p): Group(c, p) for c in range(NMC) for p in range(PAIRS)}

            # One continuous PE stream: the scores/exp pipeline never drains
            # across group boundaries (scores stay 2 (g,j)-steps ahead of
            # attnV globally) and v-projection blocks / per-chunk output
            # projections slot in as independent PE filler.
            if causal:
                group_order = [(c, p) for c in range(NMC) for p in range(PAIRS)]
                # chunk-0 scores pre-issued: PE work while xv streams in
                pre = PAIRS * 4  # scores steps emitted before the merge loop
                items = [("vp", 0, 4)]
                for c, p in group_order:
                    g = groups[(c, p)]
                    items += [("av", g, j) for j in range(g.J)]
                    last = (c, p) == group_order[-1]
                    items.append(("nmt" if last else "nm", g))
                    if (c, p) == (0, 1):
                        items.append(("vp", 4, 8))
                    elif (c, p) == (1, 0):
                        items += [("vp", 8, 12), ("op", 0)]
                    elif (c, p) == (1, 1):
                        items.append(("vp", 12, 16))
                    elif (c, p) == (2, 0):
                        items.append(("op", 1))
                    elif (c, p) == (3, 0):
                        items.append(("op", 2))
                    elif last:
                        items.append(("op", 3))
            else:
                group_order = [(c, p) for c in range(NMC) for p in range(PAIRS)]
                pre = 0
                items = [("vp", 0, NT)]
                for c, p in group_order:
                    g = groups[(c, p)]
                    items += [("av", g, j) for j in range(g.J)]
                    last = (c, p) == group_order[-1]
                    items.append(("nmt" if last else "nm", g))
                    if p == PAIRS - 1 and c > 0:
                        items.append(("op", c - 1))
                    if last:
                        items.append(("op", NMC - 1))

            sc_steps = [
                (groups[(c, p)], j)
                for c, p in group_order
                for j in range(groups[(c, p)].J)
            ]
            for gg, j in sc_steps[:pre]:
                gg.emit_scores(j)
            si = pre
            av_k = 0
            for item in items:
                if item[0] == "av":
                    while si < min(av_k + 3, len(sc_steps)):
                        sg, sj = sc_steps[si]
                        sg.emit_scores(sj)
                        si += 1
                    item[1].emit_attnv(item[2])
                    av_k += 1
                elif item[0] == "nm":
                    item[1].emit_norm()
                elif item[0] == "nmt":
                    item[1].emit_norm_transpose()
                elif item[0] == "vp":
                    emit_vproj(item[1], item[2])
                else:
                    emit_outproj(item[1], tail=(item[1] == NMC - 1))

    nc.compile()
    return nc


def _host_inputs(key, value, query, Wk, Wq, Wv, Wo, bq, bk, bv, bias_qk, bias_v):
    """Per-core input maps (host-side shard/transpose/cast — not timed)."""
    tri = np.triu(np.ones((P, P), np.float32)).astype(BF16)  # allowed: n<=m
    in_maps = []
    xT = {}
    for b in range(B):
        xT[("q", b)] = np.ascontiguousarray(query[b].T).astype(BF16)
        xT[("k", b)] = np.ascontiguousarray(key[b].T).astype(BF16)
        xT[("v", b)] = np.ascontiguousarray(value[b].T).astype(BF16)
    for c in range(NCORES):
        b, g = divmod(c, G)
        sl = slice(DKH * g, DKH * (g + 1))
        wqv = np.concatenate([Wq[sl].T, Wv[sl].T], axis=1)  # [E, 2*DKH]
        m = {
            "xqT": xT[("q", b)],
            "xkT": xT[("k", b)],
            "xvT": xT[("v", b)],
            "wkT": np.ascontiguousarray(Wk[sl].T).astype(BF16),
            "wqvT": np.ascontiguousarray(wqv).astype(BF16),
            "woT": np.ascontiguousarray(Wo[:, sl].T).astype(BF16),
            "dmask": tri,
        }
        if bias_qk:
            m["bq"] = np.ascontiguousarray(bq[sl].astype(np.float32).reshape(DKH, 1))
            m["bk"] = np.ascontiguousarray(bk[sl].astype(np.float32).reshape(DKH, 1))
        if bias_v:
            m["bv"] = np.ascontiguousarray(bv[sl].astype(np.float32).reshape(1, DKH))
        in_maps.append(m)
    return in_maps


def _numpy_fallback(key, value, query, mask, Wk, bk, Wq, bq, Wv, bv, Wo, bo):
    """Exact reference semantics in numpy (general-mask fallback)."""
    def proj(x, W, b):
        return x @ W.T + b

    k = proj(key, Wk, bk).reshape(B, S, H, DK).transpose(0, 2, 1, 3)
    q = proj(query, Wq, bq).reshape(B, S, H, DK).transpose(0, 2, 1, 3)
    v = proj(value, Wv, bv).reshape(B, S, H, DK).transpose(0, 2, 1, 3)
    scores = np.einsum("bhmd,bhnd->bhmn", q, k).astype(np.float32)
    scores = np.where(mask, scores, np.float32(-1e10)) * np.float32(SCALE)
    scores -= scores.max(axis=3, keepdims=True)
    e = np.exp(scores)
    attn = e / e.sum(axis=3, keepdims=True)
    o = np.einsum("bhmn,bhnv->bhmv", attn, v)
    o = o.transpose(0, 2, 1, 3).reshape(B, S, E)
    return (o @ Wo.T + bo).astype(np.float32)


_program_cache = {}


def kernel(key, value, query, mask, Wk, bk, Wq, bq, Wv, bv, Wo, bo):
    key = np.asarray(key, np.float32)
    value = np.asarray(value, np.float32)
    query = np.asarray(query, np.float32)
    mask = np.asarray(mask)
    Wk, bk = np.asarray(Wk, np.float32), np.asarray(bk, np.float32)
    Wq, bq = np.asarray(Wq, np.float32), np.asarray(bq, np.float32)
    Wv, bv = np.asarray(Wv, np.float32), np.asarray(bv, np.float32)
    Wo, bo = np.asarray(Wo, np.float32), np.asarray(bo, np.float32)

    m2 = mask.reshape(B, S, S) if mask.size == B * S * S else None
    causal = m2 is not None and all(
        np.array_equal(m2[b], np.tril(np.ones((S, S), bool))) for b in range(B)
    )
    allones = m2 is not None and bool(mask.all())
    if not causal and not allones:
        return _numpy_fallback(key, value, query, mask, Wk, bk, Wq, bq, Wv, bv, Wo, bo)

    if causal:
        chunk_ntiles = tuple(4 * (c + 1) for c in range(NMC))
    else:
        chunk_ntiles = tuple(NT for _ in range(NMC))

    bias_qk = bool(np.any(bq) or np.any(bk))
    bias_v = bool(np.any(bv))

    pkey = (chunk_ntiles, causal, bias_qk, bias_v)
    if pkey not in _program_cache:
        _program_cache[pkey] = _build_program(chunk_ntiles, causal, bias_qk, bias_v)
    nc = _program_cache[pkey]

    from concourse.bass_utils import run_bass_kernel_spmd

    in_maps = _host_inputs(key, value, query, Wk, Wq, Wv, Wo, bq, bk, bv, bias_qk, bias_v)
    res = run_bass_kernel_spmd(nc, in_maps, core_ids=list(range(NCORES)))

    outp = np.zeros((B, S, E), np.float32)
    for c in range(NCORES):
        outp[c // G] += np.asarray(res.results[c]["out"], np.float32)
    outp += bo.astype(np.float32)
    return outp
